# revision 13
# baseline (speedup 1.0000x reference)
"""Causal self-attention (B=4, T=2048, D=1024, H=16) on 8 trn2 NeuronCores.

Sharding: core c handles batch b=c//2 and head-group hg=c%2 (8 of 16 heads).
W_q/W_k/W_v are column-sharded per head-group (host-side). After attention,
each pair of cores AllGathers the transposed attention output (channels) and
computes a disjoint half of the output channels of the O-projection
(W_o.T column-sharded per rank parity); the host concatenates the halves.

v3 schedule:
- Inputs host-packed into wide-row tiles (8KB DMA rows): wq/wk/wv/wo as
  [128, 4096] (d-tile-major columns), x as [128, 4*4096] (chunk-major,
  d-tile-minor) so each 512-query chunk arrives as one 1MB transfer.
- Work order is J-outer: (J, hp) for J in 0..3, hp in 0..3. All first-half
  AllGathers complete early, so the first 3/4 of the output projection is
  interleaved as tensor-engine filler into the exp-bound late iterations.
- Diagonal score tiles are N-trimmed (queries < keys are never computed).
- AllGather split per head-pair: A=[0,T/2) after row J=1, B=[T/2,3T/4)
  after J=2, C=[3T/4,T) after J=3. The last two iterations self-drain
  their AV work so their seg-C AllGathers issue as early as possible, and
  the last O-proj group pre-accumulates the 6 already-gathered channel
  tiles into SBUF while the final AllGather is in flight (only 2 matmuls
  + an add remain after it lands).
- Single flat pool scope (one drain round at exit); y written bf16 and
  upcast host-side.

All matmuls run in bf16 with fp32 PSUM accumulation. Softmax is computed
without max-subtraction (scores are O(1); exp is safe), with the
denominator obtained from an extra ones-column appended to V.
"""

import os
import sys

for _p in ("/opt/trn_rl_repo", "/root/.axon_site/_ro/trn_rl_repo"):
    if os.path.isdir(_p) and _p not in sys.path:
        sys.path.insert(0, _p)

import ml_dtypes
import numpy as np

import concourse.bass as bass  # noqa: F401  (AP helpers)
import concourse.mybir as mybir
import concourse.tile as tile
from concourse.bacc import Bacc
from concourse.bass_utils import run_bass_kernel_spmd
from concourse.masks import make_identity

B = 4
D = 1024
H = 16
DH = 64
N_CORES = 8
HG = 2              # tensor-parallel degree within a batch (head groups)
CL = D // HG        # 512 local channels (8 heads) per core
SCALE = 1.0 / 8.0   # 1 / sqrt(DH)

BF16 = mybir.dt.bfloat16
F32 = mybir.dt.float32
NPBF16 = ml_dtypes.bfloat16
EXP = mybir.ActivationFunctionType.Exp

T_FULL = 2048


def build_nc(T):
    assert T == 2048, "v3 schedule is specialized to T=2048"
    NT = T // 128          # t-tiles (16)
    ND = D // 128          # d-tiles (8)
    NCT = CL // 128        # local c-tiles / head pairs (4)
    NJ = T // 512          # tq chunks (4)
    TH = T // 2            # 1024
    TQ = T // 4            # 512

    nc = Bacc(None)
    # packed inputs: 8KB rows for near-peak DMA
    xp = nc.dram_tensor("xp", [128, NJ * ND * 512], BF16, kind="ExternalInput")
    wqp = nc.dram_tensor("wqp", [128, ND * 512], BF16, kind="ExternalInput")
    wkp = nc.dram_tensor("wkp", [128, ND * 512], BF16, kind="ExternalInput")
    wvp = nc.dram_tensor("wvp", [128, ND * 512], BF16, kind="ExternalInput")
    wop = nc.dram_tensor("wop", [128, ND * 512], BF16, kind="ExternalInput")
    mask = nc.dram_tensor("mask", [128, 128], BF16, kind="ExternalInput")
    y = nc.dram_tensor("y", [T, CL], BF16, kind="ExternalOutput")

    def xsl(dt, lo, hi):
        """x slice AP: columns [lo, hi) of d-tile dt (global q index)."""
        c0, c1 = lo // 512, (hi - 1) // 512
        assert c0 == c1, "x slice must stay within one 512-chunk"
        base = c0 * (ND * 512) + dt * 512 + (lo - c0 * 512)
        return x_sb[:, base:base + (hi - lo)]

    with tile.TileContext(nc) as tc:
        with (
            tc.tile_pool(name="const", bufs=1) as constp,
            tc.tile_pool(name="wox", bufs=1) as woxp,
            tc.tile_pool(name="qk", bufs=1) as qkp,
            tc.tile_pool(name="vaug", bufs=1) as vaugp,
            tc.tile_pool(name="outT", bufs=1) as outTp,
            tc.tile_pool(name="agsb", bufs=1) as agsbp,
            tc.tile_pool(name="att", bufs=26) as attp,
            tc.tile_pool(name="on", bufs=3) as onp,
            tc.tile_pool(name="rc", bufs=4) as rcp,
            tc.tile_pool(name="ysb", bufs=2) as ysbp,
            tc.tile_pool(name="ypart", bufs=1) as ypartp,
            tc.tile_pool(name="dram", bufs=1, space="DRAM") as dramp,
            tc.tile_pool(name="acc", bufs=2, space="PSUM") as accp,
            tc.tile_pool(name="stps", bufs=2, space="PSUM") as stpsp,
            tc.tile_pool(name="avps", bufs=2, space="PSUM") as avpsp,
        ):
            mask_sb = constp.tile([128, 128], BF16, tag="mask", name="maskt")
            nc.sync.dma_start(mask_sb[:], mask[:])
            ident = constp.tile([128, 128], BF16, tag="ident", name="ident")
            make_identity(nc, ident[:])

            wq_sb = woxp.tile([128, ND * 512], BF16, tag="wq", name="wq")
            wk_sb = woxp.tile([128, ND * 512], BF16, tag="wk", name="wk")
            wv_sb = woxp.tile([128, ND * 512], BF16, tag="wv", name="wv")
            wo_sb = woxp.tile([128, ND * 512], BF16, tag="wo", name="wo")
            x_sb = woxp.tile([128, NJ * ND * 512], BF16, tag="x", name="x")

            # DMA order: wq, x chunk 0, wk, wv, x chunks 1-3, wo.
            nc.sync.dma_start(wq_sb[:], wqp[:])
            nc.sync.dma_start(x_sb[:, 0:ND * 512], xp[:, 0:ND * 512])
            nc.sync.dma_start(wk_sb[:], wkp[:])
            nc.sync.dma_start(wv_sb[:], wvp[:])
            for c in range(1, NJ):
                cs = slice(c * ND * 512, (c + 1) * ND * 512)
                nc.sync.dma_start(x_sb[:, cs], xp[:, cs])
            nc.sync.dma_start(wo_sb[:], wop[:])

            qt_sb = [qkp.tile([128, T], BF16, tag=f"q{ct}", name=f"q{ct}") for ct in range(NCT)]
            kt_sb = [qkp.tile([128, T], BF16, tag=f"k{ct}", name=f"k{ct}") for ct in range(NCT)]
            vaug_sb = [vaugp.tile([128, 8 * 65], BF16, tag=f"v{tt}", name=f"v{tt}") for tt in range(NT)]
            outT_sb = [outTp.tile([128, T], BF16, tag=f"o{ct}", name=f"o{ct}") for ct in range(NCT)]

            # AllGather DRAM staging: per hp, segment A=[0,TH), B=[TH,TH+TQ),
            # C=[TH+TQ, T)
            seg_cols = {"A": (0, TH), "B": (TH, TH + TQ), "C": (TH + TQ, T)}
            ag_in = {}
            ag_out = {}
            for hp in range(NCT):
                for s, (lo, hi) in seg_cols.items():
                    w = hi - lo
                    ag_in[(hp, s)] = dramp.tile([128, w], BF16, tag=f"agi{hp}{s}",
                                                name=f"agi{hp}{s}")
                    ag_out[(hp, s)] = dramp.tile([256, w], BF16, tag=f"ago{hp}{s}",
                                                 name=f"ago{hp}{s}")

            # PE warmup on the shared acc bank: keep the systolic array
            # active through the initial DMA window (HAM at K=8/8).
            junk = constp.tile([128, 512], BF16, tag="junk", name="junk")
            nc.vector.memset(junk[:], 0.5)
            wps = accp.tile([128, 512], F32, tag="acc", name="accw")

            def emit_junk(n):
                for _ in range(n):
                    nc.tensor.matmul(wps[:], junk[:, 0:128], junk[:],
                                     start=True, stop=True)

            emit_junk(14)

            # ---- QKV emit helpers ----
            def emit_qt(w_sb, dst, ct, tq):
                ps = accp.tile([128, 512], F32, tag="acc", name="accq")
                for dt in range(ND):
                    nc.tensor.matmul(
                        ps[:],
                        w_sb[:, dt * 512 + ct * 128:dt * 512 + (ct + 1) * 128],
                        xsl(dt, tq * 512, (tq + 1) * 512),
                        start=(dt == 0), stop=(dt == ND - 1),
                    )
                nc.vector.tensor_copy(dst[ct][:, tq * 512:(tq + 1) * 512], ps[:])

            def emit_v(tt):
                ps = accp.tile([128, 512], F32, tag="acc", name="accv")
                for dt in range(ND):
                    nc.tensor.matmul(
                        ps[:],
                        xsl(dt, tt * 128, (tt + 1) * 128),
                        wv_sb[:, dt * 512:(dt + 1) * 512],
                        start=(dt == 0), stop=(dt == ND - 1),
                    )
                nc.vector.memset(vaug_sb[tt][:], 1.0)
                dst = vaug_sb[tt][:].rearrange("p (h e) -> p h e", e=65)[:, :, 0:64]
                src = ps[:].rearrange("p (h e) -> p h e", e=64)
                nc.vector.tensor_copy(dst, src)

            # upfront: only what iteration (J=0, hp=0) needs for scores
            emit_qt(wq_sb, qt_sb, 0, 0)
            emit_qt(wk_sb, kt_sb, 0, 0)

            # deferred QKV work, tagged with the J-outer iteration index
            # (J*NCT + hp) that first consumes it; junk fillers plug the
            # early DMA-paced idle windows.
            fillers = []  # (deadline_idx, closure)
            fillers.append((1, lambda: emit_junk(6)))
            fillers.append((2, lambda: emit_junk(6)))
            for tt in range(NT):
                fillers.append(((tt // 4) * NCT + 1,
                                lambda tt=tt: emit_v(tt)))
            for c in range(NJ):
                for ct in range(NCT):
                    if c == 0 and ct == 0:
                        continue
                    fillers.append(
                        (c * NCT + ct, lambda ct=ct, c=c: emit_qt(wq_sb, qt_sb, ct, c)))
                    fillers.append(
                        (c * NCT + ct, lambda ct=ct, c=c: emit_qt(wk_sb, kt_sb, ct, c)))

            # ---------------- Attention + interleaved O-proj ----------------
            def emit_qk_tile(hp, J, i, atts):
                st = stpsp.tile([128, 1024], F32, tag="st", name="st")
                k = i - 4 * J
                o = max(k, 0) * 128  # N-trim: queries < keys are dead
                for h in range(2):
                    nc.tensor.matmul(
                        st[:, h * 512 + o:(h + 1) * 512],
                        kt_sb[hp][h * 64:(h + 1) * 64, i * 128:(i + 1) * 128],
                        qt_sb[hp][h * 64:(h + 1) * 64, J * 512 + o:(J + 1) * 512],
                        start=True, stop=True, tile_position=(h * 64, 0),
                    )
                att = attp.tile([128, 1024], BF16, tag="att", name="att")
                if o == 0:
                    nc.scalar.activation(att[:, 0:1024], st[:, 0:1024], EXP, scale=SCALE)
                else:
                    st3 = st[:].rearrange("p (h q) -> p h q", h=2)[:, :, o:512]
                    at3 = att[:].rearrange("p (h q) -> p h q", h=2)[:, :, o:512]
                    nc.scalar.activation(at3, st3, EXP, scale=SCALE)
                if k >= 0:  # diagonal 128-block: keep tk <= tq
                    for h in range(2):
                        lo = h * 512 + k * 128
                        nc.vector.tensor_mul(
                            att[:, lo:lo + 128], att[:, lo:lo + 128], mask_sb[:]
                        )
                atts.append(att)

            def emit_av_mms(hp, J, jj, h, av, atts):
                jq = 4 * J + jj
                for i in range(jq + 1):
                    lhsT = atts[i][:, h * 512 + jj * 128:h * 512 + (jj + 1) * 128]
                    hl = hp * 2 + h
                    nc.tensor.matmul(
                        av[:, h * 65:(h + 1) * 65],
                        lhsT,
                        vaug_sb[i][:, hl * 65:(hl + 1) * 65],
                        start=(i == 0), stop=(i == jq),
                    )

            def emit_av_finish(hp, J, jj, av):
                onorm = onp.tile([128, 128], BF16, tag="on", name="on")
                for h in range(2):
                    rc = rcp.tile([128, 1], F32, tag="rc", name="rc")
                    nc.vector.reciprocal(rc[:], av[:, h * 65 + 64:h * 65 + 65])
                    nc.vector.tensor_scalar_mul(
                        onorm[:, h * 64:(h + 1) * 64],
                        av[:, h * 65:h * 65 + 64],
                        rc[:],
                    )
                # transpose scratch folded into the av bank (bf16 view of
                # the same PSUM tile; av is fully consumed by the reads
                # above)
                tp = av.bitcast(BF16)[:, 0:128]
                nc.tensor.transpose(tp, onorm[:], ident[:])
                nc.vector.tensor_copy(
                    outT_sb[hp][:, J * 512 + jj * 128:J * 512 + (jj + 1) * 128],
                    tp,
                )

            def emit_ag(hp, seg):
                lo, hi = seg_cols[seg]
                nc.gpsimd.dma_start(
                    ag_in[(hp, seg)][:], outT_sb[hp][:, lo:hi])
                nc.gpsimd.collective_compute(
                    "AllGather",
                    mybir.AluOpType.bypass,
                    replica_groups=[[0, 1], [2, 3], [4, 5], [6, 7]],
                    ins=[ag_in[(hp, seg)].opt()],
                    outs=[ag_out[(hp, seg)].opt()],
                )

            def make_av_items(hp, J, atts):
                items = []
                for jj in range(4):
                    av = avpsp.tile([128, 130], F32, tag="av", name="av")
                    for h in range(2):
                        items.append(
                            lambda hp=hp, J=J, jj=jj, h=h, av=av, atts=atts:
                            emit_av_mms(hp, J, jj, h, av, atts)
                        )
                    items.append(
                        lambda hp=hp, J=J, jj=jj, av=av:
                        emit_av_finish(hp, J, jj, av)
                    )
                return items

            def emit_av_self(hp, J, jj, atts):
                av = avpsp.tile([128, 130], F32, tag="av", name="av")
                for h in range(2):
                    emit_av_mms(hp, J, jj, h, av, atts)
                emit_av_finish(hp, J, jj, av)

            # ---- O-projection as per-t-tile units ----
            ag_cur = [None] * ND

            def load_ag_grp(grp, ct):
                """Load ag columns [grp*512, (grp+1)*512) for channel tile
                ct into the rotating slot."""
                t = agsbp.tile([128, TQ], BF16, tag=f"ag{ct}", name=f"ag{ct}")
                hp = ct % NCT
                rows = slice(0, 128) if ct < NCT else slice(128, 256)
                lo = grp * TQ
                off = 0
                for s, (slo, shi) in seg_cols.items():
                    a, b = max(lo, slo), min(lo + TQ, shi)
                    if a < b:
                        nc.sync.dma_start(
                            t[:, off:off + (b - a)],
                            ag_out[(hp, s)][rows, a - slo:b - slo])
                        off += b - a
                assert off == TQ
                ag_cur[ct] = t

            PART_CTS = [0, 4, 1, 5, 2, 6]  # hp 0-2: gathered before the tail
            TAIL_CTS = [3, 7]              # hp 3: gated by the last AG

            def oproj_unit(tt):
                yps = accp.tile([128, 512], F32, tag="acc", name="accy")
                th = tt % 4
                for cti in range(ND):
                    nc.tensor.matmul(
                        yps[:],
                        ag_cur[cti][:, th * 128:(th + 1) * 128],
                        wo_sb[:, cti * 512:(cti + 1) * 512],
                        start=(cti == 0), stop=(cti == ND - 1),
                    )
                ysb = ysbp.tile([128, 512], BF16, tag="ysb", name="ysb")
                nc.vector.tensor_copy(ysb[:], yps[:])
                nc.sync.dma_start(y[tt * 128:(tt + 1) * 128, :], ysb[:])

            ypart_sb = [None] * 4

            def oproj_partial(tt):
                """Accumulate the 6 non-hp3 channel tiles of a grp3 unit
                into SBUF while the last AllGather is in flight."""
                yps = accp.tile([128, 512], F32, tag="acc", name="accp")
                th = tt % 4
                for n, cti in enumerate(PART_CTS):
                    nc.tensor.matmul(
                        yps[:],
                        ag_cur[cti][:, th * 128:(th + 1) * 128],
                        wo_sb[:, cti * 512:(cti + 1) * 512],
                        start=(n == 0), stop=(n == len(PART_CTS) - 1),
                    )
                t = ypartp.tile([128, 512], BF16, tag=f"yp{th}", name=f"yp{th}")
                nc.vector.tensor_copy(t[:], yps[:])
                ypart_sb[th] = t

            def oproj_final(tt):
                yps = accp.tile([128, 512], F32, tag="acc", name="accf")
                th = tt % 4
                for n, cti in enumerate(TAIL_CTS):
                    nc.tensor.matmul(
                        yps[:],
                        ag_cur[cti][:, th * 128:(th + 1) * 128],
                        wo_sb[:, cti * 512:(cti + 1) * 512],
                        start=(n == 0), stop=(n == len(TAIL_CTS) - 1),
                    )
                ysb = ysbp.tile([128, 512], BF16, tag="ysb", name="ysb")
                nc.vector.tensor_add(ysb[:], yps[:], ypart_sb[th][:])
                nc.sync.dma_start(y[tt * 128:(tt + 1) * 128, :], ysb[:])

            # O-proj fillers: (deadline, closure). Group g covers t-tiles
            # 4g..4g+3; loads precede units at equal deadlines (stable
            # sort). grp3: non-hp3 loads land in idx 15 after grp2 units;
            # partial/tail work runs post-loop.
            oitems = []
            for ct in range(ND):
                oitems.append((9, lambda ct=ct: load_ag_grp(0, ct)))
            for tt in range(0, 4):
                oitems.append((10 + tt // 2, lambda tt=tt: oproj_unit(tt)))
            for ct in range(ND):
                oitems.append((11, lambda ct=ct: load_ag_grp(1, ct)))
            for tt in range(4, 8):
                oitems.append((12 + (tt - 4) // 2, lambda tt=tt: oproj_unit(tt)))
            for ct in range(ND):
                oitems.append((13, lambda ct=ct: load_ag_grp(2, ct)))
            for tt in range(8, 12):
                oitems.append((14 + (tt - 8) // 2, lambda tt=tt: oproj_unit(tt)))
            for ct in PART_CTS:
                oitems.append((15, lambda ct=ct: load_ag_grp(3, ct)))
            oitems.sort(key=lambda x: x[0])

            work = [(J, hp) for J in range(NJ) for hp in range(NCT)]
            n_work = len(work)
            av_queue = []
            fillers.sort(key=lambda x: x[0])
            fpos = 0
            opos = 0
            for idx, (J, hp) in enumerate(work):
                selfdrain = idx >= n_work - 2
                n_tk = 4 * J + 4
                atts = []
                due = []
                while fpos < len(fillers) and fillers[fpos][0] <= idx + 1:
                    due.append(fillers[fpos][1])
                    fpos += 1
                while opos < len(oitems) and oitems[opos][0] <= idx:
                    due.append(oitems[opos][1])
                    opos += 1
                mixed = []
                na, nd_ = len(av_queue), len(due)
                ai = di = 0
                for s in range(na + nd_):
                    if ai * nd_ <= di * na and ai < na:
                        mixed.append(av_queue[ai]); ai += 1
                    elif di < nd_:
                        mixed.append(due[di]); di += 1
                    else:
                        mixed.append(av_queue[ai]); ai += 1
                total = len(mixed)
                done = 0
                for i in range(n_tk):
                    emit_qk_tile(hp, J, i, atts)
                    want = ((i + 1) * total) // n_tk
                    while done < want:
                        mixed[done]()
                        done += 1
                    # self-drain: AV(jj) emitted one tile after its last
                    # att (one-tile slack for the exp pipeline)
                    if selfdrain and i >= n_tk - 3:
                        emit_av_self(hp, J, i - (n_tk - 3), atts)
                while done < total:
                    mixed[done]()
                    done += 1
                if selfdrain:
                    emit_av_self(hp, J, 3, atts)
                    emit_ag(hp, "C")
                    av_queue = []
                else:
                    av_queue = make_av_items(hp, J, atts)
                    if J == NJ // 2 - 1:
                        av_queue.append(lambda hp=hp: emit_ag(hp, "A"))
                    elif J == NJ - 2:
                        av_queue.append(lambda hp=hp: emit_ag(hp, "B"))
                    elif J == NJ - 1:
                        av_queue.append(lambda hp=hp: emit_ag(hp, "C"))
            for c in av_queue:
                c()
            # tail: pre-accumulate grp3 partials while AG(3,C) is in
            # flight, then the 2-matmul finishes.
            for tt in range(12, 16):
                oproj_partial(tt)
            for ct in TAIL_CTS:
                load_ag_grp(3, ct)
            for tt in range(12, 16):
                oproj_final(tt)

    nc.compile()
    return nc


_NC_CACHE = {}


def _get_nc(T):
    if T not in _NC_CACHE:
        _NC_CACHE[T] = build_nc(T)
    return _NC_CACHE[T]


def _pack_w(w):
    """[1024, 512] (in-dim major) -> [128, 8*512] d-tile-major columns."""
    return np.ascontiguousarray(
        w.reshape(8, 128, 512).transpose(1, 0, 2).reshape(128, 8 * 512))


def _pack_x(xT):
    """[1024, T] -> [128, NJ*8*512] chunk-major, d-tile-minor."""
    T = xT.shape[1]
    nj = T // 512
    return np.ascontiguousarray(
        xT.reshape(8, 128, nj, 512).transpose(1, 2, 0, 3).reshape(128, nj * 8 * 512))


def shard_inputs(x, W_q, W_k, W_v, W_o):
    """Host-side sharding: per-core input dicts (bf16, packed)."""
    tri = np.triu(np.ones((128, 128), np.float32)).astype(NPBF16)
    xp_cache = {}
    in_maps = []
    for c in range(N_CORES):
        b, hg = c // 2, c % 2
        cs = slice(hg * CL, (hg + 1) * CL)
        if b not in xp_cache:
            xp_cache[b] = _pack_x(np.ascontiguousarray(x[b].T).astype(NPBF16))
        in_maps.append({
            "xp": xp_cache[b],
            "wqp": _pack_w(np.ascontiguousarray(W_q[cs, :].T).astype(NPBF16)),
            "wkp": _pack_w(np.ascontiguousarray(W_k[cs, :].T).astype(NPBF16)),
            "wvp": _pack_w(np.ascontiguousarray(W_v[cs, :].T).astype(NPBF16)),
            "wop": _pack_w(np.ascontiguousarray(W_o[cs, :].T).astype(NPBF16)),
            "mask": tri,
        })
    return in_maps


def assemble_output(results, T):
    y = np.zeros((B, T, D), np.float32)
    for c in range(N_CORES):
        b, hg = c // 2, c % 2
        y[b][:, hg * CL:(hg + 1) * CL] = results[c]["y"].astype(np.float32)
    return y


def kernel(x, W_q, W_k, W_v, W_o, _trace=False):
    x = np.asarray(x, dtype=np.float32)
    W_q = np.asarray(W_q, dtype=np.float32)
    W_k = np.asarray(W_k, dtype=np.float32)
    W_v = np.asarray(W_v, dtype=np.float32)
    W_o = np.asarray(W_o, dtype=np.float32)
    T = x.shape[1]
    nc = _get_nc(T)
    in_maps = shard_inputs(x, W_q, W_k, W_v, W_o)
    res = run_bass_kernel_spmd(
        nc, in_maps, core_ids=list(range(N_CORES)), trace=_trace
    )
    out = assemble_output(res.results, T)
    if _trace:
        return out, res
    return out


# revision 25
# speedup vs baseline: 1.0938x; 1.0938x over previous
"""Causal self-attention (B=4, T=2048, D=1024, H=16) on 8 trn2 NeuronCores.

Sharding: core c handles batch b=c//2 and head-group hg=c%2 (8 of 16 heads).
W_q/W_k/W_v are column-sharded per head-group (host-side). After attention,
each pair of cores AllGathers the transposed attention output (channels) and
computes a disjoint half of the output channels of the O-projection
(W_o.T column-sharded per rank parity); the host concatenates the halves.

v3 schedule:
- Inputs host-packed into wide-row tiles (8KB DMA rows): wq/wk/wv/wo as
  [128, 4096] (d-tile-major columns), x as [128, 4*4096] (chunk-major,
  d-tile-minor) so each 512-query chunk arrives as one 1MB transfer.
- Work order is J-outer: (J, hp) for J in 0..3, hp in 0..3. All first-half
  AllGathers complete early, so the first 3/4 of the output projection is
  interleaved as tensor-engine filler into the exp-bound late iterations.
- Diagonal score tiles are N-trimmed (queries < keys are never computed).
- AllGather split per head-pair: A=[0,T/2) after row J=1, B=[T/2,3T/4)
  after J=2, C=[3T/4,T) after J=3. The last two iterations self-drain
  their AV work so their seg-C AllGathers issue as early as possible, and
  the last O-proj group pre-accumulates the 6 already-gathered channel
  tiles into SBUF while the final AllGather is in flight (only 2 matmuls
  + an add remain after it lands).
- Single flat pool scope (one drain round at exit); y written bf16 and
  upcast host-side.

All matmuls run in bf16 with fp32 PSUM accumulation. Softmax is computed
without max-subtraction (scores are O(1); exp is safe), with the
denominator obtained from an extra ones-column appended to V.
"""

import os
import sys

for _p in ("/opt/trn_rl_repo", "/root/.axon_site/_ro/trn_rl_repo"):
    if os.path.isdir(_p) and _p not in sys.path:
        sys.path.insert(0, _p)

import ml_dtypes
import numpy as np

import concourse.bass as bass  # noqa: F401  (AP helpers)
import concourse.mybir as mybir
import concourse.tile as tile
from concourse.bacc import Bacc
from concourse.bass_utils import run_bass_kernel_spmd
from concourse.masks import make_identity

B = 4
D = 1024
H = 16
DH = 64
N_CORES = 8
HG = 2              # tensor-parallel degree within a batch (head groups)
CL = D // HG        # 512 local channels (8 heads) per core
SCALE = 1.0 / 8.0   # 1 / sqrt(DH)

BF16 = mybir.dt.bfloat16
F32 = mybir.dt.float32
NPBF16 = ml_dtypes.bfloat16
EXP = mybir.ActivationFunctionType.Exp

T_FULL = 2048


def build_nc(T):
    assert T == 2048, "v3 schedule is specialized to T=2048"
    NT = T // 128          # t-tiles (16)
    ND = D // 128          # d-tiles (8)
    NCT = CL // 128        # local c-tiles / head pairs (4)
    NJ = T // 512          # tq chunks (4)
    TH = T // 2            # 1024
    TQ = T // 4            # 512

    nc = Bacc(None)
    # packed inputs: 8KB rows for near-peak DMA
    xp = nc.dram_tensor("xp", [128, NJ * ND * 512], BF16, kind="ExternalInput")
    wqp = nc.dram_tensor("wqp", [128, ND * 512], BF16, kind="ExternalInput")
    wkp = nc.dram_tensor("wkp", [128, ND * 512], BF16, kind="ExternalInput")
    wvp = nc.dram_tensor("wvp", [128, ND * 512], BF16, kind="ExternalInput")
    wop = nc.dram_tensor("wop", [128, ND * 512], BF16, kind="ExternalInput")
    mask = nc.dram_tensor("mask", [128, 128], BF16, kind="ExternalInput")
    y = nc.dram_tensor("y", [T, CL], BF16, kind="ExternalOutput")

    def xsl(dt, lo, hi):
        """x slice AP: columns [lo, hi) of d-tile dt (global q index)."""
        c0, c1 = lo // 512, (hi - 1) // 512
        assert c0 == c1, "x slice must stay within one 512-chunk"
        base = c0 * (ND * 512) + dt * 512 + (lo - c0 * 512)
        return x_sb[:, base:base + (hi - lo)]

    with tile.TileContext(nc) as tc:
        with (
            tc.tile_pool(name="const", bufs=1) as constp,
            tc.tile_pool(name="wox", bufs=1) as woxp,
            tc.tile_pool(name="qk", bufs=1) as qkp,
            tc.tile_pool(name="vaug", bufs=1) as vaugp,
            tc.tile_pool(name="outT", bufs=1) as outTp,
            tc.tile_pool(name="agsb", bufs=2) as agsbp,
            tc.tile_pool(name="att", bufs=24) as attp,
            tc.tile_pool(name="on", bufs=3) as onp,
            tc.tile_pool(name="rc", bufs=4) as rcp,
            tc.tile_pool(name="ysb", bufs=2) as ysbp,
            tc.tile_pool(name="ypart", bufs=1) as ypartp,
            tc.tile_pool(name="dram", bufs=1, space="DRAM") as dramp,
            tc.tile_pool(name="acc", bufs=2, space="PSUM") as accp,
            tc.tile_pool(name="stps", bufs=2, space="PSUM") as stpsp,
            tc.tile_pool(name="avps", bufs=2, space="PSUM") as avpsp,
        ):
            mask_sb = constp.tile([128, 128], BF16, tag="mask", name="maskt")
            nc.sync.dma_start(mask_sb[:], mask[:])
            ident = constp.tile([128, 128], BF16, tag="ident", name="ident")
            make_identity(nc, ident[:])

            wq_sb = woxp.tile([128, ND * 512], BF16, tag="wq", name="wq")
            wk_sb = woxp.tile([128, ND * 512], BF16, tag="wk", name="wk")
            wv_sb = woxp.tile([128, ND * 512], BF16, tag="wv", name="wv")
            wo_sb = woxp.tile([128, ND * 512], BF16, tag="wo", name="wo")
            x_sb = woxp.tile([128, NJ * ND * 512], BF16, tag="x", name="x")

            # DMA order: wq, x chunk 0, wk, wv, x chunks 1-3, wo.
            nc.sync.dma_start(wq_sb[:], wqp[:])
            nc.sync.dma_start(x_sb[:, 0:ND * 512], xp[:, 0:ND * 512])
            nc.sync.dma_start(wk_sb[:], wkp[:])
            nc.sync.dma_start(wv_sb[:], wvp[:])
            for c in range(1, NJ):
                cs = slice(c * ND * 512, (c + 1) * ND * 512)
                nc.sync.dma_start(x_sb[:, cs], xp[:, cs])
            nc.sync.dma_start(wo_sb[:], wop[:])

            qt_sb = [qkp.tile([128, T], BF16, tag=f"q{ct}", name=f"q{ct}") for ct in range(NCT)]
            kt_sb = [qkp.tile([128, T], BF16, tag=f"k{ct}", name=f"k{ct}") for ct in range(NCT)]
            vaug_sb = [vaugp.tile([128, 8 * 65], BF16, tag=f"v{tt}", name=f"v{tt}") for tt in range(NT)]
            outT_sb = [outTp.tile([128, T], BF16, tag=f"o{ct}", name=f"o{ct}") for ct in range(NCT)]

            # Bundled AllGathers (collectives have a multi-us fixed cost and
            # serialize on one CC stream, so bundle across head-pairs):
            #   A01/A23: outT cols [0,TH) of hp pairs, after rows J=0..1
            #   B: cols [TH,TH+TQ) of all hp, after row J=2
            #   C012: cols [TH+TQ,T) of hp 0-2; C3: same cols of hp 3
            ag_bundles = {
                "A01": ([0, 1], 0, TH),
                "A23": ([2, 3], 0, TH),
                "B": ([0, 1, 2, 3], TH, TH + TQ),
                "C012": ([0, 1, 2], TH + TQ, T),
                "C3": ([3], TH + TQ, T),
            }
            ag_in = {}
            ag_out = {}
            for s, (hps, lo, hi) in ag_bundles.items():
                w = (hi - lo) * len(hps)
                ag_in[s] = dramp.tile([128, w], BF16, tag=f"agi{s}",
                                      name=f"agi{s}")
                ag_out[s] = dramp.tile([256, w], BF16, tag=f"ago{s}",
                                       name=f"ago{s}")

            # PE warmup on the shared acc bank: keep the systolic array
            # active through the initial DMA window (HAM at K=8/8).
            junk = constp.tile([128, 512], BF16, tag="junk", name="junk")
            nc.vector.memset(junk[:], 0.5)
            wps = accp.tile([128, 512], F32, tag="acc", name="accw")

            def emit_junk(n):
                for _ in range(n):
                    nc.tensor.matmul(wps[:], junk[:, 0:128], junk[:],
                                     start=True, stop=True)

            emit_junk(14)

            # ---- QKV emit helpers ----
            def emit_qt(w_sb, dst, ct, tq):
                ps = accp.tile([128, 512], F32, tag="acc", name="accq")
                for dt in range(ND):
                    nc.tensor.matmul(
                        ps[:],
                        w_sb[:, dt * 512 + ct * 128:dt * 512 + (ct + 1) * 128],
                        xsl(dt, tq * 512, (tq + 1) * 512),
                        start=(dt == 0), stop=(dt == ND - 1),
                    )
                nc.vector.tensor_copy(dst[ct][:, tq * 512:(tq + 1) * 512], ps[:])

            def emit_v(tt):
                ps = accp.tile([128, 512], F32, tag="acc", name="accv")
                for dt in range(ND):
                    nc.tensor.matmul(
                        ps[:],
                        xsl(dt, tt * 128, (tt + 1) * 128),
                        wv_sb[:, dt * 512:(dt + 1) * 512],
                        start=(dt == 0), stop=(dt == ND - 1),
                    )
                nc.vector.memset(vaug_sb[tt][:], 1.0)
                dst = vaug_sb[tt][:].rearrange("p (h e) -> p h e", e=65)[:, :, 0:64]
                src = ps[:].rearrange("p (h e) -> p h e", e=64)
                nc.vector.tensor_copy(dst, src)

            # upfront: only what iteration (J=0, hp=0) needs for scores
            emit_qt(wq_sb, qt_sb, 0, 0)
            emit_qt(wk_sb, kt_sb, 0, 0)

            # deferred QKV work, tagged with the J-outer iteration index
            # (J*NCT + hp) that first consumes it; junk fillers plug the
            # early DMA-paced idle windows.
            fillers = []  # (deadline_idx, closure)
            fillers.append((1, lambda: emit_junk(6)))
            fillers.append((2, lambda: emit_junk(6)))
            for tt in range(NT):
                fillers.append(((tt // 4) * NCT + 1,
                                lambda tt=tt: emit_v(tt)))
            for c in range(NJ):
                for ct in range(NCT):
                    if c == 0 and ct == 0:
                        continue
                    fillers.append(
                        (c * NCT + ct, lambda ct=ct, c=c: emit_qt(wq_sb, qt_sb, ct, c)))
                    fillers.append(
                        (c * NCT + ct, lambda ct=ct, c=c: emit_qt(wk_sb, kt_sb, ct, c)))

            # ---------------- Attention + interleaved O-proj ----------------
            def emit_qk_tile(hp, J, i, atts):
                st = stpsp.tile([128, 1024], F32, tag="st", name="st")
                k = i - 4 * J
                o = max(k, 0) * 128  # N-trim: queries < keys are dead
                for h in range(2):
                    nc.tensor.matmul(
                        st[:, h * 512 + o:(h + 1) * 512],
                        kt_sb[hp][h * 64:(h + 1) * 64, i * 128:(i + 1) * 128],
                        qt_sb[hp][h * 64:(h + 1) * 64, J * 512 + o:(J + 1) * 512],
                        start=True, stop=True, tile_position=(h * 64, 0),
                    )
                att = attp.tile([128, 1024], BF16, tag="att", name="att")
                if o == 0:
                    nc.scalar.activation(att[:, 0:1024], st[:, 0:1024], EXP, scale=SCALE)
                else:
                    st3 = st[:].rearrange("p (h q) -> p h q", h=2)[:, :, o:512]
                    at3 = att[:].rearrange("p (h q) -> p h q", h=2)[:, :, o:512]
                    nc.scalar.activation(at3, st3, EXP, scale=SCALE)
                if k >= 0:  # diagonal 128-block: keep tk <= tq
                    for h in range(2):
                        lo = h * 512 + k * 128
                        nc.vector.tensor_mul(
                            att[:, lo:lo + 128], att[:, lo:lo + 128], mask_sb[:]
                        )
                atts.append(att)

            def emit_av_mms(hp, J, jj, h, av, atts):
                jq = 4 * J + jj
                for i in range(jq + 1):
                    lhsT = atts[i][:, h * 512 + jj * 128:h * 512 + (jj + 1) * 128]
                    hl = hp * 2 + h
                    nc.tensor.matmul(
                        av[:, h * 65:(h + 1) * 65],
                        lhsT,
                        vaug_sb[i][:, hl * 65:(hl + 1) * 65],
                        start=(i == 0), stop=(i == jq),
                    )

            def emit_av_finish(hp, J, jj, av):
                onorm = onp.tile([128, 128], BF16, tag="on", name="on")
                for h in range(2):
                    rc = rcp.tile([128, 1], F32, tag="rc", name="rc")
                    nc.vector.reciprocal(rc[:], av[:, h * 65 + 64:h * 65 + 65])
                    nc.vector.tensor_scalar_mul(
                        onorm[:, h * 64:(h + 1) * 64],
                        av[:, h * 65:h * 65 + 64],
                        rc[:],
                    )
                # transpose scratch folded into the av bank (bf16 view of
                # the same PSUM tile; av is fully consumed by the reads
                # above)
                tp = av.bitcast(BF16)[:, 0:128]
                nc.tensor.transpose(tp, onorm[:], ident[:])
                nc.vector.tensor_copy(
                    outT_sb[hp][:, J * 512 + jj * 128:J * 512 + (jj + 1) * 128],
                    tp,
                )

            def emit_ag(seg):
                hps, lo, hi = ag_bundles[seg]
                w = hi - lo
                for n, hp in enumerate(hps):
                    nc.gpsimd.dma_start(
                        ag_in[seg][:, n * w:(n + 1) * w], outT_sb[hp][:, lo:hi])
                nc.gpsimd.collective_compute(
                    "AllGather",
                    mybir.AluOpType.bypass,
                    replica_groups=[[0, 1], [2, 3], [4, 5], [6, 7]],
                    ins=[ag_in[seg].opt()],
                    outs=[ag_out[seg].opt()],
                )

            def make_av_items(hp, J, atts):
                items = []
                for jj in range(4):
                    av = avpsp.tile([128, 130], F32, tag="av", name="av")
                    for h in range(2):
                        items.append(
                            lambda hp=hp, J=J, jj=jj, h=h, av=av, atts=atts:
                            emit_av_mms(hp, J, jj, h, av, atts)
                        )
                    items.append(
                        lambda hp=hp, J=J, jj=jj, av=av:
                        emit_av_finish(hp, J, jj, av)
                    )
                return items

            def emit_av_self(hp, J, jj, atts):
                av = avpsp.tile([128, 130], F32, tag="av", name="av")
                for h in range(2):
                    emit_av_mms(hp, J, jj, h, av, atts)
                emit_av_finish(hp, J, jj, av)

            # ---- O-projection as per-t-tile units ----
            ag_cur = [None] * ND

            def load_ag_grp(grp, ct):
                """Load ag columns [grp*512, (grp+1)*512) for channel tile
                ct into the rotating slot."""
                t = agsbp.tile([128, TQ], BF16, tag=f"ag{ct}", name=f"ag{ct}")
                hp = ct % NCT
                rows = slice(0, 128) if ct < NCT else slice(128, 256)
                lo = grp * TQ
                # find the bundle holding [lo, lo+TQ) for this hp
                for s, (hps, slo, shi) in ag_bundles.items():
                    if hp in hps and slo <= lo and lo + TQ <= shi:
                        n = hps.index(hp)
                        w = shi - slo
                        nc.sync.dma_start(
                            t[:],
                            ag_out[s][rows, n * w + (lo - slo):
                                      n * w + (lo - slo) + TQ])
                        break
                else:
                    raise AssertionError(f"no bundle for grp={grp} ct={ct}")
                return t

            PART_CTS = [0, 4, 1, 5, 2, 6]  # hp 0-2: gathered before the tail
            TAIL_CTS = [3, 7]              # hp 3: gated by the last AG

            def oproj_unit(tt):
                yps = accp.tile([128, 512], F32, tag="acc", name="accy")
                th = tt % 4
                for cti in range(ND):
                    nc.tensor.matmul(
                        yps[:],
                        ag_cur[cti][:, th * 128:(th + 1) * 128],
                        wo_sb[:, cti * 512:(cti + 1) * 512],
                        start=(cti == 0), stop=(cti == ND - 1),
                    )
                ysb = ysbp.tile([128, 512], BF16, tag="ysb", name="ysb")
                nc.vector.tensor_copy(ysb[:], yps[:])
                nc.sync.dma_start(y[tt * 128:(tt + 1) * 128, :], ysb[:])

            ypart_sb = [None] * 4

            def oproj_partial(tt):
                """Accumulate the 6 non-hp3 channel tiles of a grp3 unit
                into SBUF while the last AllGather is in flight."""
                yps = accp.tile([128, 512], F32, tag="acc", name="accp")
                th = tt % 4
                for n, cti in enumerate(PART_CTS):
                    nc.tensor.matmul(
                        yps[:],
                        ag3[cti][:, th * 128:(th + 1) * 128],
                        wo_sb[:, cti * 512:(cti + 1) * 512],
                        start=(n == 0), stop=(n == len(PART_CTS) - 1),
                    )
                t = ypartp.tile([128, 512], BF16, tag=f"yp{th}", name=f"yp{th}")
                nc.vector.tensor_copy(t[:], yps[:])
                ypart_sb[th] = t

            def oproj_final(tt):
                yps = accp.tile([128, 512], F32, tag="acc", name="accf")
                th = tt % 4
                for n, cti in enumerate(TAIL_CTS):
                    nc.tensor.matmul(
                        yps[:],
                        ag3[cti][:, th * 128:(th + 1) * 128],
                        wo_sb[:, cti * 512:(cti + 1) * 512],
                        start=(n == 0), stop=(n == len(TAIL_CTS) - 1),
                    )
                ysb = ysbp.tile([128, 512], BF16, tag="ysb", name="ysb")
                nc.vector.tensor_add(ysb[:], yps[:], ypart_sb[th][:])
                nc.sync.dma_start(y[tt * 128:(tt + 1) * 128, :], ysb[:])

            # O-proj fillers: (deadline, closure). Group g covers t-tiles
            # 4g..4g+3; loads precede units at equal deadlines (stable
            # sort). grp3: non-hp3 loads land in idx 15 after grp2 units;
            # partial/tail work runs post-loop.
            def load_ag_cur(grp, ct):
                ag_cur[ct] = load_ag_grp(grp, ct)

            ag3 = [None] * ND

            def load_ag3(ct):
                ag3[ct] = load_ag_grp(3, ct)

            oitems = []
            for ct in range(ND):
                oitems.append((10, lambda ct=ct: load_ag_cur(0, ct)))
            for tt in range(0, 4):
                oitems.append((11 + tt // 2, lambda tt=tt: oproj_unit(tt)))
            for ct in range(ND):
                oitems.append((12, lambda ct=ct: load_ag_cur(1, ct)))
            for tt in range(4, 8):
                oitems.append((13 + (tt - 4) // 2, lambda tt=tt: oproj_unit(tt)))
            for ct in range(ND):
                oitems.append((15, lambda ct=ct: load_ag_cur(2, ct)))
            oitems.sort(key=lambda x: x[0])

            work = [(J, hp) for J in range(NJ) for hp in range(NCT)]
            n_work = len(work)
            av_queue = []
            fillers.sort(key=lambda x: x[0])
            fpos = 0
            opos = 0
            for idx, (J, hp) in enumerate(work):
                selfdrain = idx >= n_work - 2
                n_tk = 4 * J + 4
                atts = []
                due = []
                while fpos < len(fillers) and fillers[fpos][0] <= idx + 1:
                    due.append(fillers[fpos][1])
                    fpos += 1
                while opos < len(oitems) and oitems[opos][0] <= idx:
                    due.append(oitems[opos][1])
                    opos += 1
                mixed = []
                na, nd_ = len(av_queue), len(due)
                ai = di = 0
                for s in range(na + nd_):
                    if ai * nd_ <= di * na and ai < na:
                        mixed.append(av_queue[ai]); ai += 1
                    elif di < nd_:
                        mixed.append(due[di]); di += 1
                    else:
                        mixed.append(av_queue[ai]); ai += 1
                total = len(mixed)
                done = 0
                for i in range(n_tk):
                    emit_qk_tile(hp, J, i, atts)
                    want = ((i + 1) * total) // n_tk
                    while done < want:
                        mixed[done]()
                        done += 1
                    # self-drain: AV(jj) emitted one tile after its last
                    # att (one-tile slack for the exp pipeline)
                    if selfdrain and i >= n_tk - 3:
                        emit_av_self(hp, J, i - (n_tk - 3), atts)
                while done < total:
                    mixed[done]()
                    done += 1
                if selfdrain:
                    emit_av_self(hp, J, 3, atts)
                    emit_ag("C012" if idx == n_work - 2 else "C3")
                    av_queue = []
                else:
                    av_queue = make_av_items(hp, J, atts)
                # bundled AG issue points (staging DMAs wait on the outT
                # writes via semaphores, so issuing at iteration end of the
                # last contributing drain is safe)
                if idx == 6:
                    emit_ag("A01")
                elif idx == 8:
                    emit_ag("A23")
                elif idx == 12:
                    emit_ag("B")
            for c in av_queue:
                c()
            # tail: grp3 partial loads (waiting on C012) first, then grp2
            # units + grp3 partials interleave while C3 is in flight; the
            # 2-matmul finishes run last.
            for ct in PART_CTS:
                load_ag3(ct)
            for tt in range(12, 16):
                oproj_partial(tt)
                oproj_unit(tt - 4)
            for ct in TAIL_CTS:
                load_ag3(ct)
            for tt in range(12, 16):
                oproj_final(tt)

    nc.compile()
    return nc


_NC_CACHE = {}


def _get_nc(T):
    if T not in _NC_CACHE:
        _NC_CACHE[T] = build_nc(T)
    return _NC_CACHE[T]


def _pack_w(w):
    """[1024, 512] (in-dim major) -> [128, 8*512] d-tile-major columns."""
    return np.ascontiguousarray(
        w.reshape(8, 128, 512).transpose(1, 0, 2).reshape(128, 8 * 512))


def _pack_x(xT):
    """[1024, T] -> [128, NJ*8*512] chunk-major, d-tile-minor."""
    T = xT.shape[1]
    nj = T // 512
    return np.ascontiguousarray(
        xT.reshape(8, 128, nj, 512).transpose(1, 2, 0, 3).reshape(128, nj * 8 * 512))


def shard_inputs(x, W_q, W_k, W_v, W_o):
    """Host-side sharding: per-core input dicts (bf16, packed)."""
    tri = np.triu(np.ones((128, 128), np.float32)).astype(NPBF16)
    xp_cache = {}
    in_maps = []
    for c in range(N_CORES):
        b, hg = c // 2, c % 2
        cs = slice(hg * CL, (hg + 1) * CL)
        if b not in xp_cache:
            xp_cache[b] = _pack_x(np.ascontiguousarray(x[b].T).astype(NPBF16))
        in_maps.append({
            "xp": xp_cache[b],
            "wqp": _pack_w(np.ascontiguousarray(W_q[cs, :].T).astype(NPBF16)),
            "wkp": _pack_w(np.ascontiguousarray(W_k[cs, :].T).astype(NPBF16)),
            "wvp": _pack_w(np.ascontiguousarray(W_v[cs, :].T).astype(NPBF16)),
            "wop": _pack_w(np.ascontiguousarray(W_o[cs, :].T).astype(NPBF16)),
            "mask": tri,
        })
    return in_maps


def assemble_output(results, T):
    y = np.zeros((B, T, D), np.float32)
    for c in range(N_CORES):
        b, hg = c // 2, c % 2
        y[b][:, hg * CL:(hg + 1) * CL] = results[c]["y"].astype(np.float32)
    return y


def kernel(x, W_q, W_k, W_v, W_o, _trace=False):
    x = np.asarray(x, dtype=np.float32)
    W_q = np.asarray(W_q, dtype=np.float32)
    W_k = np.asarray(W_k, dtype=np.float32)
    W_v = np.asarray(W_v, dtype=np.float32)
    W_o = np.asarray(W_o, dtype=np.float32)
    T = x.shape[1]
    nc = _get_nc(T)
    in_maps = shard_inputs(x, W_q, W_k, W_v, W_o)
    res = run_bass_kernel_spmd(
        nc, in_maps, core_ids=list(range(N_CORES)), trace=_trace
    )
    out = assemble_output(res.results, T)
    if _trace:
        return out, res
    return out


# revision 30
# speedup vs baseline: 1.1077x; 1.0127x over previous
"""Causal self-attention (B=4, T=2048, D=1024, H=16) on 8 trn2 NeuronCores.

Sharding: core c handles batch b=c//2 and head-group hg=c%2 (8 of 16 heads).
W_q/W_k/W_v are column-sharded per head-group (host-side). After attention,
each pair of cores AllGathers the transposed attention output (channels) and
computes a disjoint half of the output channels of the O-projection
(W_o.T column-sharded per rank parity); the host concatenates the halves.

v3 schedule:
- Inputs host-packed into wide-row tiles (8KB DMA rows): wq/wk/wv/wo as
  [128, 4096] (d-tile-major columns), x as [128, 4*4096] (chunk-major,
  d-tile-minor) so each 512-query chunk arrives as one 1MB transfer.
- Work order is J-outer: (J, hp) for J in 0..3, hp in 0..3. All first-half
  AllGathers complete early, so the first 3/4 of the output projection is
  interleaved as tensor-engine filler into the exp-bound late iterations.
- Diagonal score tiles are N-trimmed (queries < keys are never computed).
- AllGather split per head-pair: A=[0,T/2) after row J=1, B=[T/2,3T/4)
  after J=2, C=[3T/4,T) after J=3. The last two iterations self-drain
  their AV work so their seg-C AllGathers issue as early as possible, and
  the last O-proj group pre-accumulates the 6 already-gathered channel
  tiles into SBUF while the final AllGather is in flight (only 2 matmuls
  + an add remain after it lands).
- Single flat pool scope (one drain round at exit); y written bf16 and
  upcast host-side.

All matmuls run in bf16 with fp32 PSUM accumulation. Softmax is computed
without max-subtraction (scores are O(1); exp is safe), with the
denominator obtained from an extra ones-column appended to V.
"""

import os
import sys

for _p in ("/opt/trn_rl_repo", "/root/.axon_site/_ro/trn_rl_repo"):
    if os.path.isdir(_p) and _p not in sys.path:
        sys.path.insert(0, _p)

import ml_dtypes
import numpy as np

import concourse.bass as bass  # noqa: F401  (AP helpers)
import concourse.mybir as mybir
import concourse.tile as tile
from concourse.bacc import Bacc
from concourse.bass_utils import run_bass_kernel_spmd
from concourse.masks import make_identity

B = 4
D = 1024
H = 16
DH = 64
N_CORES = 8
HG = 2              # tensor-parallel degree within a batch (head groups)
CL = D // HG        # 512 local channels (8 heads) per core
SCALE = 1.0 / 8.0   # 1 / sqrt(DH)

BF16 = mybir.dt.bfloat16
F32 = mybir.dt.float32
NPBF16 = ml_dtypes.bfloat16
EXP = mybir.ActivationFunctionType.Exp

T_FULL = 2048


def build_nc(T):
    assert T == 2048, "v3 schedule is specialized to T=2048"
    NT = T // 128          # t-tiles (16)
    ND = D // 128          # d-tiles (8)
    NCT = CL // 128        # local c-tiles / head pairs (4)
    NJ = T // 512          # tq chunks (4)
    TH = T // 2            # 1024
    TQ = T // 4            # 512

    nc = Bacc(None)
    # packed inputs: 8KB rows for near-peak DMA
    xp = nc.dram_tensor("xp", [128, NJ * ND * 512], BF16, kind="ExternalInput")
    wqp = nc.dram_tensor("wqp", [128, ND * 512], BF16, kind="ExternalInput")
    wkp = nc.dram_tensor("wkp", [128, ND * 512], BF16, kind="ExternalInput")
    wvp = nc.dram_tensor("wvp", [128, ND * 512], BF16, kind="ExternalInput")
    wop = nc.dram_tensor("wop", [128, ND * 512], BF16, kind="ExternalInput")
    mask = nc.dram_tensor("mask", [128, 128], BF16, kind="ExternalInput")
    y = nc.dram_tensor("y", [T, CL], BF16, kind="ExternalOutput")

    def xsl(dt, lo, hi):
        """x slice AP: columns [lo, hi) of d-tile dt (global q index)."""
        c0, c1 = lo // 512, (hi - 1) // 512
        assert c0 == c1, "x slice must stay within one 512-chunk"
        base = c0 * (ND * 512) + dt * 512 + (lo - c0 * 512)
        return x_sb[:, base:base + (hi - lo)]

    with tile.TileContext(nc) as tc:
        with (
            tc.tile_pool(name="const", bufs=1) as constp,
            tc.tile_pool(name="wox", bufs=1) as woxp,
            tc.tile_pool(name="qk", bufs=1) as qkp,
            tc.tile_pool(name="vaug", bufs=1) as vaugp,
            tc.tile_pool(name="outT", bufs=1) as outTp,
            tc.tile_pool(name="agsb", bufs=2) as agsbp,
            tc.tile_pool(name="att", bufs=24) as attp,
            tc.tile_pool(name="on", bufs=3) as onp,
            tc.tile_pool(name="rc", bufs=4) as rcp,
            tc.tile_pool(name="ysb", bufs=2) as ysbp,
            tc.tile_pool(name="ypart", bufs=1) as ypartp,
            tc.tile_pool(name="dram", bufs=1, space="DRAM") as dramp,
            tc.tile_pool(name="acc", bufs=2, space="PSUM") as accp,
            tc.tile_pool(name="stps", bufs=2, space="PSUM") as stpsp,
            tc.tile_pool(name="avps", bufs=2, space="PSUM") as avpsp,
        ):
            mask_sb = constp.tile([128, 128], BF16, tag="mask", name="maskt")
            nc.sync.dma_start(mask_sb[:], mask[:])
            ident = constp.tile([128, 128], BF16, tag="ident", name="ident")
            make_identity(nc, ident[:])

            wq_sb = woxp.tile([128, ND * 512], BF16, tag="wq", name="wq")
            wk_sb = woxp.tile([128, ND * 512], BF16, tag="wk", name="wk")
            wv_sb = woxp.tile([128, ND * 512], BF16, tag="wv", name="wv")
            wo_sb = woxp.tile([128, ND * 512], BF16, tag="wo", name="wo")
            x_sb = woxp.tile([128, NJ * ND * 512], BF16, tag="x", name="x")

            # DMA order: wq, x chunk 0, wk, wv, x chunks 1-3, wo.
            nc.sync.dma_start(wq_sb[:], wqp[:])
            nc.sync.dma_start(x_sb[:, 0:ND * 512], xp[:, 0:ND * 512])
            nc.sync.dma_start(wk_sb[:], wkp[:])
            nc.sync.dma_start(wv_sb[:], wvp[:])
            for c in range(1, NJ):
                cs = slice(c * ND * 512, (c + 1) * ND * 512)
                nc.sync.dma_start(x_sb[:, cs], xp[:, cs])
            nc.sync.dma_start(wo_sb[:], wop[:])

            qt_sb = [qkp.tile([128, T], BF16, tag=f"q{ct}", name=f"q{ct}") for ct in range(NCT)]
            kt_sb = [qkp.tile([128, T], BF16, tag=f"k{ct}", name=f"k{ct}") for ct in range(NCT)]
            vaug_sb = [vaugp.tile([128, 8 * 65], BF16, tag=f"v{tt}", name=f"v{tt}") for tt in range(NT)]
            outT_sb = [outTp.tile([128, T], BF16, tag=f"o{ct}", name=f"o{ct}") for ct in range(NCT)]

            # Bundled AllGathers (collectives have a multi-us fixed cost and
            # serialize on one CC stream, so bundle across head-pairs):
            #   A01/A23: outT cols [0,TH) of hp pairs, after rows J=0..1
            #   B: cols [TH,TH+TQ) of all hp, after row J=2
            #   C012: cols [TH+TQ,T) of hp 0-2; C3: same cols of hp 3
            ag_bundles = {
                "A01": ([0, 1], 0, TH),
                "A23": ([2, 3], 0, TH),
                "B": ([0, 1, 2, 3], TH, TH + TQ),
                "C012": ([0, 1, 2], TH + TQ, T),
                "C3": ([3], TH + TQ, T),
            }
            ag_in = {}
            ag_out = {}
            for s, (hps, lo, hi) in ag_bundles.items():
                w = (hi - lo) * len(hps)
                ag_in[s] = dramp.tile([128, w], BF16, tag=f"agi{s}",
                                      name=f"agi{s}")
                ag_out[s] = dramp.tile([256, w], BF16, tag=f"ago{s}",
                                       name=f"ago{s}")

            # PE warmup on the shared acc bank: keep the systolic array
            # active through the initial DMA window (HAM at K=8/8).
            junk = constp.tile([128, 512], BF16, tag="junk", name="junk")
            nc.vector.memset(junk[:], 0.5)
            wps = accp.tile([128, 512], F32, tag="acc", name="accw")

            def emit_junk(n):
                for _ in range(n):
                    nc.tensor.matmul(wps[:], junk[:, 0:128], junk[:],
                                     start=True, stop=True)

            emit_junk(14)

            # ---- QKV emit helpers ----
            def emit_qt(w_sb, dst, ct, tq):
                ps = accp.tile([128, 512], F32, tag="acc", name="accq")
                for dt in range(ND):
                    nc.tensor.matmul(
                        ps[:],
                        w_sb[:, dt * 512 + ct * 128:dt * 512 + (ct + 1) * 128],
                        xsl(dt, tq * 512, (tq + 1) * 512),
                        start=(dt == 0), stop=(dt == ND - 1),
                    )
                nc.vector.tensor_copy(dst[ct][:, tq * 512:(tq + 1) * 512], ps[:])

            def emit_v(tt):
                ps = accp.tile([128, 512], F32, tag="acc", name="accv")
                for dt in range(ND):
                    nc.tensor.matmul(
                        ps[:],
                        xsl(dt, tt * 128, (tt + 1) * 128),
                        wv_sb[:, dt * 512:(dt + 1) * 512],
                        start=(dt == 0), stop=(dt == ND - 1),
                    )
                nc.vector.memset(vaug_sb[tt][:], 1.0)
                dst = vaug_sb[tt][:].rearrange("p (h e) -> p h e", e=65)[:, :, 0:64]
                src = ps[:].rearrange("p (h e) -> p h e", e=64)
                nc.vector.tensor_copy(dst, src)

            # upfront: only what iteration (J=0, hp=0) needs for scores;
            # junk matmuls absorb the DMA pacing between the chains
            emit_qt(wq_sb, qt_sb, 0, 0)
            emit_junk(4)
            emit_qt(wk_sb, kt_sb, 0, 0)
            emit_junk(8)

            # deferred QKV work, tagged with the J-outer iteration index
            # (J*NCT + hp) that first consumes it; junk fillers plug the
            # early DMA-paced idle windows.
            fillers = []  # (deadline_idx, closure)
            fillers.append((1, lambda: emit_junk(6)))
            fillers.append((2, lambda: emit_junk(6)))
            for tt in range(NT):
                fillers.append(((tt // 4) * NCT + 1,
                                lambda tt=tt: emit_v(tt)))
            for c in range(NJ):
                for ct in range(NCT):
                    if c == 0 and ct == 0:
                        continue
                    fillers.append(
                        (c * NCT + ct, lambda ct=ct, c=c: emit_qt(wq_sb, qt_sb, ct, c)))
                    fillers.append(
                        (c * NCT + ct, lambda ct=ct, c=c: emit_qt(wk_sb, kt_sb, ct, c)))

            # ---------------- Attention + interleaved O-proj ----------------
            def emit_qk_tile(hp, J, i, atts):
                st = stpsp.tile([128, 1024], F32, tag="st", name="st")
                k = i - 4 * J
                o = max(k, 0) * 128  # N-trim: queries < keys are dead
                for h in range(2):
                    nc.tensor.matmul(
                        st[:, h * 512 + o:(h + 1) * 512],
                        kt_sb[hp][h * 64:(h + 1) * 64, i * 128:(i + 1) * 128],
                        qt_sb[hp][h * 64:(h + 1) * 64, J * 512 + o:(J + 1) * 512],
                        start=True, stop=True, tile_position=(h * 64, 0),
                    )
                att = attp.tile([128, 1024], BF16, tag="att", name="att")
                if o == 0:
                    nc.scalar.activation(att[:, 0:1024], st[:, 0:1024], EXP, scale=SCALE)
                else:
                    st3 = st[:].rearrange("p (h q) -> p h q", h=2)[:, :, o:512]
                    at3 = att[:].rearrange("p (h q) -> p h q", h=2)[:, :, o:512]
                    nc.scalar.activation(at3, st3, EXP, scale=SCALE)
                if k >= 0:  # diagonal 128-block: keep tk <= tq
                    for h in range(2):
                        lo = h * 512 + k * 128
                        nc.vector.tensor_mul(
                            att[:, lo:lo + 128], att[:, lo:lo + 128], mask_sb[:]
                        )
                atts.append(att)

            def emit_av_mms(hp, J, jj, h, av, atts):
                jq = 4 * J + jj
                for i in range(jq + 1):
                    lhsT = atts[i][:, h * 512 + jj * 128:h * 512 + (jj + 1) * 128]
                    hl = hp * 2 + h
                    nc.tensor.matmul(
                        av[:, h * 65:(h + 1) * 65],
                        lhsT,
                        vaug_sb[i][:, hl * 65:(hl + 1) * 65],
                        start=(i == 0), stop=(i == jq),
                    )

            def emit_av_finish(hp, J, jj, av):
                onorm = onp.tile([128, 128], BF16, tag="on", name="on")
                for h in range(2):
                    rc = rcp.tile([128, 1], F32, tag="rc", name="rc")
                    nc.vector.reciprocal(rc[:], av[:, h * 65 + 64:h * 65 + 65])
                    nc.vector.tensor_scalar_mul(
                        onorm[:, h * 64:(h + 1) * 64],
                        av[:, h * 65:h * 65 + 64],
                        rc[:],
                    )
                # transpose scratch folded into the av bank (bf16 view of
                # the same PSUM tile; av is fully consumed by the reads
                # above)
                tp = av.bitcast(BF16)[:, 0:128]
                nc.tensor.transpose(tp, onorm[:], ident[:])
                nc.vector.tensor_copy(
                    outT_sb[hp][:, J * 512 + jj * 128:J * 512 + (jj + 1) * 128],
                    tp,
                )

            def emit_ag(seg):
                hps, lo, hi = ag_bundles[seg]
                w = hi - lo
                for n, hp in enumerate(hps):
                    nc.gpsimd.dma_start(
                        ag_in[seg][:, n * w:(n + 1) * w], outT_sb[hp][:, lo:hi])
                nc.gpsimd.collective_compute(
                    "AllGather",
                    mybir.AluOpType.bypass,
                    replica_groups=[[0, 1], [2, 3], [4, 5], [6, 7]],
                    ins=[ag_in[seg].opt()],
                    outs=[ag_out[seg].opt()],
                )

            def make_av_items(hp, J, atts):
                items = []
                for jj in range(4):
                    av = avpsp.tile([128, 130], F32, tag="av", name="av")
                    for h in range(2):
                        items.append(
                            lambda hp=hp, J=J, jj=jj, h=h, av=av, atts=atts:
                            emit_av_mms(hp, J, jj, h, av, atts)
                        )
                    items.append(
                        lambda hp=hp, J=J, jj=jj, av=av:
                        emit_av_finish(hp, J, jj, av)
                    )
                return items

            def emit_av_self(hp, J, jj, atts):
                av = avpsp.tile([128, 130], F32, tag="av", name="av")
                for h in range(2):
                    emit_av_mms(hp, J, jj, h, av, atts)
                emit_av_finish(hp, J, jj, av)

            # ---- O-projection as per-t-tile units ----
            ag_cur = [None] * ND

            def load_ag_grp(grp, ct):
                """Load ag columns [grp*512, (grp+1)*512) for channel tile
                ct into the rotating slot."""
                t = agsbp.tile([128, TQ], BF16, tag=f"ag{ct}", name=f"ag{ct}")
                hp = ct % NCT
                rows = slice(0, 128) if ct < NCT else slice(128, 256)
                lo = grp * TQ
                # find the bundle holding [lo, lo+TQ) for this hp
                for s, (hps, slo, shi) in ag_bundles.items():
                    if hp in hps and slo <= lo and lo + TQ <= shi:
                        n = hps.index(hp)
                        w = shi - slo
                        nc.sync.dma_start(
                            t[:],
                            ag_out[s][rows, n * w + (lo - slo):
                                      n * w + (lo - slo) + TQ])
                        break
                else:
                    raise AssertionError(f"no bundle for grp={grp} ct={ct}")
                return t

            PART_CTS = [0, 4, 1, 5, 2, 6]  # hp 0-2: gathered before the tail
            TAIL_CTS = [3, 7]              # hp 3: gated by the last AG

            def oproj_unit(tt):
                yps = accp.tile([128, 512], F32, tag="acc", name="accy")
                th = tt % 4
                for cti in range(ND):
                    nc.tensor.matmul(
                        yps[:],
                        ag_cur[cti][:, th * 128:(th + 1) * 128],
                        wo_sb[:, cti * 512:(cti + 1) * 512],
                        start=(cti == 0), stop=(cti == ND - 1),
                    )
                ysb = ysbp.tile([128, 512], BF16, tag="ysb", name="ysb")
                nc.vector.tensor_copy(ysb[:], yps[:])
                nc.sync.dma_start(y[tt * 128:(tt + 1) * 128, :], ysb[:])

            ypart_sb = [None] * 4

            def oproj_partial(tt):
                """Accumulate the 6 non-hp3 channel tiles of a grp3 unit
                into SBUF while the last AllGather is in flight."""
                yps = accp.tile([128, 512], F32, tag="acc", name="accp")
                th = tt % 4
                for n, cti in enumerate(PART_CTS):
                    nc.tensor.matmul(
                        yps[:],
                        ag3[cti][:, th * 128:(th + 1) * 128],
                        wo_sb[:, cti * 512:(cti + 1) * 512],
                        start=(n == 0), stop=(n == len(PART_CTS) - 1),
                    )
                t = ypartp.tile([128, 512], BF16, tag=f"yp{th}", name=f"yp{th}")
                nc.vector.tensor_copy(t[:], yps[:])
                ypart_sb[th] = t

            def oproj_final(tt):
                yps = accp.tile([128, 512], F32, tag="acc", name="accf")
                th = tt % 4
                for n, cti in enumerate(TAIL_CTS):
                    nc.tensor.matmul(
                        yps[:],
                        ag3[cti][:, th * 128:(th + 1) * 128],
                        wo_sb[:, cti * 512:(cti + 1) * 512],
                        start=(n == 0), stop=(n == len(TAIL_CTS) - 1),
                    )
                ysb = ysbp.tile([128, 512], BF16, tag="ysb", name="ysb")
                nc.vector.tensor_add(ysb[:], yps[:], ypart_sb[th][:])
                nc.sync.dma_start(y[tt * 128:(tt + 1) * 128, :], ysb[:])

            # O-proj fillers: (deadline, closure). Group g covers t-tiles
            # 4g..4g+3; loads precede units at equal deadlines (stable
            # sort). grp3: non-hp3 loads land in idx 15 after grp2 units;
            # partial/tail work runs post-loop.
            def load_ag_cur(grp, ct):
                ag_cur[ct] = load_ag_grp(grp, ct)

            ag3 = [None] * ND

            def load_ag3(ct):
                ag3[ct] = load_ag_grp(3, ct)

            oitems = []
            for ct in range(ND):
                oitems.append((10, lambda ct=ct: load_ag_cur(0, ct)))
            for tt in range(0, 4):
                oitems.append((12 + tt // 2, lambda tt=tt: oproj_unit(tt)))
            for ct in range(ND):
                oitems.append((13, lambda ct=ct: load_ag_cur(1, ct)))
            for tt in range(4, 8):
                oitems.append((14 + (tt - 4) // 2, lambda tt=tt: oproj_unit(tt)))
            for ct in range(ND):
                oitems.append((15, lambda ct=ct: load_ag_cur(2, ct)))
            oitems.sort(key=lambda x: x[0])

            work = [(J, hp) for J in range(NJ) for hp in range(NCT)]
            n_work = len(work)
            av_queue = []
            fillers.sort(key=lambda x: x[0])
            fpos = 0
            opos = 0
            for idx, (J, hp) in enumerate(work):
                selfdrain = idx >= n_work - 2
                n_tk = 4 * J + 4
                atts = []
                due = []
                while fpos < len(fillers) and fillers[fpos][0] <= idx + 1:
                    due.append(fillers[fpos][1])
                    fpos += 1
                while opos < len(oitems) and oitems[opos][0] <= idx:
                    due.append(oitems[opos][1])
                    opos += 1
                mixed = []
                na, nd_ = len(av_queue), len(due)
                ai = di = 0
                for s in range(na + nd_):
                    if ai * nd_ <= di * na and ai < na:
                        mixed.append(av_queue[ai]); ai += 1
                    elif di < nd_:
                        mixed.append(due[di]); di += 1
                    else:
                        mixed.append(av_queue[ai]); ai += 1
                total = len(mixed)
                done = 0
                for i in range(n_tk):
                    emit_qk_tile(hp, J, i, atts)
                    want = ((i + 1) * total) // n_tk
                    while done < want:
                        mixed[done]()
                        done += 1
                    # self-drain: AV(jj) emitted one tile after its last
                    # att (one-tile slack for the exp pipeline)
                    if selfdrain and i >= n_tk - 3:
                        emit_av_self(hp, J, i - (n_tk - 3), atts)
                while done < total:
                    mixed[done]()
                    done += 1
                if selfdrain:
                    emit_av_self(hp, J, 3, atts)
                    emit_ag("C012" if idx == n_work - 2 else "C3")
                    av_queue = []
                else:
                    av_queue = make_av_items(hp, J, atts)
                # bundled AG issue points (staging DMAs wait on the outT
                # writes via semaphores, so issuing at iteration end of the
                # last contributing drain is safe)
                if idx == 6:
                    emit_ag("A01")
                elif idx == 8:
                    emit_ag("A23")
                elif idx == 12:
                    emit_ag("B")
            for c in av_queue:
                c()
            # tail: all grp3 loads issue up front on the sync queue (PART
            # ones fire immediately off the landed C012; TAIL ones wait on
            # C3), then grp2 units + grp3 partials interleave while C3 is
            # in flight; the 2-matmul finishes run last.
            for ct in PART_CTS:
                load_ag3(ct)
            for ct in TAIL_CTS:
                load_ag3(ct)
            for tt in range(12, 16):
                oproj_partial(tt)
                oproj_unit(tt - 4)
            for tt in range(12, 16):
                oproj_final(tt)

    nc.compile()
    return nc


_NC_CACHE = {}


def _get_nc(T):
    if T not in _NC_CACHE:
        _NC_CACHE[T] = build_nc(T)
    return _NC_CACHE[T]


def _pack_w(w):
    """[1024, 512] (in-dim major) -> [128, 8*512] d-tile-major columns."""
    return np.ascontiguousarray(
        w.reshape(8, 128, 512).transpose(1, 0, 2).reshape(128, 8 * 512))


def _pack_x(xT):
    """[1024, T] -> [128, NJ*8*512] chunk-major, d-tile-minor."""
    T = xT.shape[1]
    nj = T // 512
    return np.ascontiguousarray(
        xT.reshape(8, 128, nj, 512).transpose(1, 2, 0, 3).reshape(128, nj * 8 * 512))


def shard_inputs(x, W_q, W_k, W_v, W_o):
    """Host-side sharding: per-core input dicts (bf16, packed)."""
    tri = np.triu(np.ones((128, 128), np.float32)).astype(NPBF16)
    xp_cache = {}
    in_maps = []
    for c in range(N_CORES):
        b, hg = c // 2, c % 2
        cs = slice(hg * CL, (hg + 1) * CL)
        if b not in xp_cache:
            xp_cache[b] = _pack_x(np.ascontiguousarray(x[b].T).astype(NPBF16))
        in_maps.append({
            "xp": xp_cache[b],
            "wqp": _pack_w(np.ascontiguousarray(W_q[cs, :].T).astype(NPBF16)),
            "wkp": _pack_w(np.ascontiguousarray(W_k[cs, :].T).astype(NPBF16)),
            "wvp": _pack_w(np.ascontiguousarray(W_v[cs, :].T).astype(NPBF16)),
            "wop": _pack_w(np.ascontiguousarray(W_o[cs, :].T).astype(NPBF16)),
            "mask": tri,
        })
    return in_maps


def assemble_output(results, T):
    y = np.zeros((B, T, D), np.float32)
    for c in range(N_CORES):
        b, hg = c // 2, c % 2
        y[b][:, hg * CL:(hg + 1) * CL] = results[c]["y"].astype(np.float32)
    return y


def kernel(x, W_q, W_k, W_v, W_o, _trace=False):
    x = np.asarray(x, dtype=np.float32)
    W_q = np.asarray(W_q, dtype=np.float32)
    W_k = np.asarray(W_k, dtype=np.float32)
    W_v = np.asarray(W_v, dtype=np.float32)
    W_o = np.asarray(W_o, dtype=np.float32)
    T = x.shape[1]
    nc = _get_nc(T)
    in_maps = shard_inputs(x, W_q, W_k, W_v, W_o)
    res = run_bass_kernel_spmd(
        nc, in_maps, core_ids=list(range(N_CORES)), trace=_trace
    )
    out = assemble_output(res.results, T)
    if _trace:
        return out, res
    return out


# revision 35
# speedup vs baseline: 1.1433x; 1.0321x over previous
"""Causal self-attention (B=4, T=2048, D=1024, H=16) on 8 trn2 NeuronCores.

Sharding: core c handles batch b=c//2 and head-group hg=c%2 (8 of 16 heads).
W_q/W_k/W_v are column-sharded per head-group (host-side). After attention,
each pair of cores AllGathers the transposed attention output (channels) and
computes a disjoint half of the output channels of the O-projection
(W_o.T column-sharded per rank parity); the host concatenates the halves.

v3 schedule:
- Inputs host-packed into wide-row tiles (8KB DMA rows): wq/wk/wv/wo as
  [128, 4096] (d-tile-major columns), x as [128, 4*4096] (chunk-major,
  d-tile-minor) so each 512-query chunk arrives as one 1MB transfer.
- Work order is J-outer: (J, hp) for J in 0..3, hp in 0..3. All first-half
  AllGathers complete early, so the first 3/4 of the output projection is
  interleaved as tensor-engine filler into the exp-bound late iterations.
- Diagonal score tiles are N-trimmed (queries < keys are never computed).
- AllGather split per head-pair: A=[0,T/2) after row J=1, B=[T/2,3T/4)
  after J=2, C=[3T/4,T) after J=3. The last two iterations self-drain
  their AV work so their seg-C AllGathers issue as early as possible, and
  the last O-proj group pre-accumulates the 6 already-gathered channel
  tiles into SBUF while the final AllGather is in flight (only 2 matmuls
  + an add remain after it lands).
- Single flat pool scope (one drain round at exit); y written bf16 and
  upcast host-side.

All matmuls run in bf16 with fp32 PSUM accumulation. Softmax is computed
without max-subtraction (scores are O(1); exp is safe), with the
denominator obtained from an extra ones-column appended to V.
"""

import os
import sys

for _p in ("/opt/trn_rl_repo", "/root/.axon_site/_ro/trn_rl_repo"):
    if os.path.isdir(_p) and _p not in sys.path:
        sys.path.insert(0, _p)

import ml_dtypes
import numpy as np

import concourse.bass as bass  # noqa: F401  (AP helpers)
import concourse.mybir as mybir
import concourse.tile as tile
from concourse.bacc import Bacc
from concourse.bass_utils import run_bass_kernel_spmd
from concourse.masks import make_identity

B = 4
D = 1024
H = 16
DH = 64
N_CORES = 8
HG = 2              # tensor-parallel degree within a batch (head groups)
CL = D // HG        # 512 local channels (8 heads) per core
SCALE = 1.0 / 8.0   # 1 / sqrt(DH)

BF16 = mybir.dt.bfloat16
F32 = mybir.dt.float32
NPBF16 = ml_dtypes.bfloat16
EXP = mybir.ActivationFunctionType.Exp

T_FULL = 2048


def build_nc(T):
    assert T == 2048, "v3 schedule is specialized to T=2048"
    NT = T // 128          # t-tiles (16)
    ND = D // 128          # d-tiles (8)
    NCT = CL // 128        # local c-tiles / head pairs (4)
    NJ = T // 512          # tq chunks (4)
    TH = T // 2            # 1024
    TQ = T // 4            # 512

    nc = Bacc(None)
    # packed inputs: 8KB rows for near-peak DMA
    xp = nc.dram_tensor("xp", [128, NJ * ND * 512], BF16, kind="ExternalInput")
    wqp = nc.dram_tensor("wqp", [128, ND * 512], BF16, kind="ExternalInput")
    wkp = nc.dram_tensor("wkp", [128, ND * 512], BF16, kind="ExternalInput")
    wvp = nc.dram_tensor("wvp", [128, ND * 512], BF16, kind="ExternalInput")
    wop = nc.dram_tensor("wop", [128, ND * 512], BF16, kind="ExternalInput")
    mask = nc.dram_tensor("mask", [128, 128], BF16, kind="ExternalInput")
    y = nc.dram_tensor("y", [T, CL], BF16, kind="ExternalOutput")

    def xsl(dt, lo, hi):
        """x slice AP: columns [lo, hi) of d-tile dt (global q index)."""
        c0, c1 = lo // 512, (hi - 1) // 512
        assert c0 == c1, "x slice must stay within one 512-chunk"
        base = c0 * (ND * 512) + dt * 512 + (lo - c0 * 512)
        return x_sb[:, base:base + (hi - lo)]

    with tile.TileContext(nc) as tc:
        with (
            tc.tile_pool(name="const", bufs=1) as constp,
            tc.tile_pool(name="wox", bufs=1) as woxp,
            tc.tile_pool(name="qk", bufs=1) as qkp,
            tc.tile_pool(name="vaug", bufs=1) as vaugp,
            tc.tile_pool(name="outT", bufs=1) as outTp,
            tc.tile_pool(name="agsb", bufs=2) as agsbp,
            tc.tile_pool(name="att", bufs=24) as attp,
            tc.tile_pool(name="on", bufs=3) as onp,
            tc.tile_pool(name="rc", bufs=4) as rcp,
            tc.tile_pool(name="ysb", bufs=2) as ysbp,
            tc.tile_pool(name="ypart", bufs=1) as ypartp,
            tc.tile_pool(name="dram", bufs=1, space="DRAM") as dramp,
            tc.tile_pool(name="acc", bufs=2, space="PSUM") as accp,
            tc.tile_pool(name="stps", bufs=2, space="PSUM") as stpsp,
            tc.tile_pool(name="avps", bufs=2, space="PSUM") as avpsp,
        ):
            mask_sb = constp.tile([128, 128], BF16, tag="mask", name="maskt")
            nc.sync.dma_start(mask_sb[:], mask[:])
            ident = constp.tile([128, 128], BF16, tag="ident", name="ident")
            make_identity(nc, ident[:])

            wq_sb = woxp.tile([128, ND * 512], BF16, tag="wq", name="wq")
            wk_sb = woxp.tile([128, ND * 512], BF16, tag="wk", name="wk")
            wv_sb = woxp.tile([128, ND * 512], BF16, tag="wv", name="wv")
            wo_sb = woxp.tile([128, ND * 512], BF16, tag="wo", name="wo")
            x_sb = woxp.tile([128, NJ * ND * 512], BF16, tag="x", name="x")

            # DMA order: wq, x chunk 0, wk, wv, x chunks 1-3, wo.
            nc.sync.dma_start(wq_sb[:], wqp[:])
            nc.sync.dma_start(x_sb[:, 0:ND * 512], xp[:, 0:ND * 512])
            nc.sync.dma_start(wk_sb[:], wkp[:])
            nc.sync.dma_start(wv_sb[:], wvp[:])
            for c in range(1, NJ):
                cs = slice(c * ND * 512, (c + 1) * ND * 512)
                nc.sync.dma_start(x_sb[:, cs], xp[:, cs])
            nc.sync.dma_start(wo_sb[:], wop[:])

            qt_sb = [qkp.tile([128, T], BF16, tag=f"q{ct}", name=f"q{ct}") for ct in range(NCT)]
            kt_sb = [qkp.tile([128, T], BF16, tag=f"k{ct}", name=f"k{ct}") for ct in range(NCT)]
            vaug_sb = [vaugp.tile([128, 8 * 65], BF16, tag=f"v{tt}", name=f"v{tt}") for tt in range(NT)]
            outT_sb = [outTp.tile([128, T], BF16, tag=f"o{ct}", name=f"o{ct}") for ct in range(NCT)]

            # Bundled AllGathers (collectives have a multi-us fixed cost and
            # serialize on one CC stream, so bundle across head-pairs):
            #   A01/A23: outT cols [0,TH) of hp pairs, after rows J=0..1
            #   B: cols [TH,TH+TQ) of all hp, after row J=2
            #   C012: cols [TH+TQ,T) of hp 0-2; C3: same cols of hp 3
            ag_bundles = {
                "A01": ([0, 1], 0, TH),
                "A23": ([2, 3], 0, TH),
                "B": ([0, 1, 2, 3], TH, TH + TQ),
                "C012": ([0, 1, 2], TH + TQ, T),
                "C3": ([3], TH + TQ, T),
            }
            ag_in = {}
            ag_out = {}
            for s, (hps, lo, hi) in ag_bundles.items():
                w = (hi - lo) * len(hps)
                ag_in[s] = dramp.tile([128, w], BF16, tag=f"agi{s}",
                                      name=f"agi{s}")
                ag_out[s] = dramp.tile([256, w], BF16, tag=f"ago{s}",
                                       name=f"ago{s}")

            # PE warmup on the shared acc bank: keep the systolic array
            # active through the initial DMA window (HAM at K=8/8).
            junk = constp.tile([128, 512], BF16, tag="junk", name="junk")
            nc.vector.memset(junk[:], 0.5)

            def emit_junk(n):
                ps = accp.tile([128, 512], F32, tag="acc", name="accw")
                for _ in range(n):
                    nc.tensor.matmul(ps[:], junk[:, 0:128], junk[:],
                                     start=True, stop=True)

            emit_junk(14)

            # ---- QKV emit helpers ----
            def emit_qt(w_sb, dst, ct, tq):
                ps = accp.tile([128, 512], F32, tag="acc", name="accq")
                for dt in range(ND):
                    nc.tensor.matmul(
                        ps[:],
                        w_sb[:, dt * 512 + ct * 128:dt * 512 + (ct + 1) * 128],
                        xsl(dt, tq * 512, (tq + 1) * 512),
                        start=(dt == 0), stop=(dt == ND - 1),
                    )
                nc.vector.tensor_copy(dst[ct][:, tq * 512:(tq + 1) * 512], ps[:])

            def emit_v(tt):
                ps = accp.tile([128, 512], F32, tag="acc", name="accv")
                for dt in range(ND):
                    nc.tensor.matmul(
                        ps[:],
                        xsl(dt, tt * 128, (tt + 1) * 128),
                        wv_sb[:, dt * 512:(dt + 1) * 512],
                        start=(dt == 0), stop=(dt == ND - 1),
                    )
                nc.vector.memset(vaug_sb[tt][:], 1.0)
                dst = vaug_sb[tt][:].rearrange("p (h e) -> p h e", e=65)[:, :, 0:64]
                src = ps[:].rearrange("p (h e) -> p h e", e=64)
                nc.vector.tensor_copy(dst, src)

            # upfront: only what iteration (J=0, hp=0) needs for scores;
            # junk matmuls absorb the DMA pacing between the chains
            emit_qt(wq_sb, qt_sb, 0, 0)
            emit_junk(4)
            emit_qt(wk_sb, kt_sb, 0, 0)
            emit_junk(8)

            # deferred QKV work, tagged with the J-outer iteration index
            # (J*NCT + hp) that first consumes it; junk fillers plug the
            # early DMA-paced idle windows.
            fillers = []  # (deadline_idx, closure)
            fillers.append((1, lambda: emit_junk(6)))
            fillers.append((2, lambda: emit_junk(6)))
            # HAM keep-warm for the exp-bound self-drain iterations
            for dl in (14, 14, 15, 15):
                fillers.append((dl, lambda: emit_junk(3)))
            for tt in range(NT):
                fillers.append(((tt // 4) * NCT + 1,
                                lambda tt=tt: emit_v(tt)))
            for c in range(NJ):
                for ct in range(NCT):
                    if c == 0 and ct == 0:
                        continue
                    fillers.append(
                        (c * NCT + ct, lambda ct=ct, c=c: emit_qt(wq_sb, qt_sb, ct, c)))
                    fillers.append(
                        (c * NCT + ct, lambda ct=ct, c=c: emit_qt(wk_sb, kt_sb, ct, c)))

            # ---------------- Attention + interleaved O-proj ----------------
            def emit_qk_tile(hp, J, i, atts):
                st = stpsp.tile([128, 1024], F32, tag="st", name="st")
                k = i - 4 * J
                o = max(k, 0) * 128  # N-trim: queries < keys are dead
                for h in range(2):
                    nc.tensor.matmul(
                        st[:, h * 512 + o:(h + 1) * 512],
                        kt_sb[hp][h * 64:(h + 1) * 64, i * 128:(i + 1) * 128],
                        qt_sb[hp][h * 64:(h + 1) * 64, J * 512 + o:(J + 1) * 512],
                        start=True, stop=True, tile_position=(h * 64, 0),
                    )
                att = attp.tile([128, 1024], BF16, tag="att", name="att")
                if o == 0:
                    nc.scalar.activation(att[:, 0:1024], st[:, 0:1024], EXP, scale=SCALE)
                else:
                    st3 = st[:].rearrange("p (h q) -> p h q", h=2)[:, :, o:512]
                    at3 = att[:].rearrange("p (h q) -> p h q", h=2)[:, :, o:512]
                    nc.scalar.activation(at3, st3, EXP, scale=SCALE)
                if k >= 0:  # diagonal 128-block: keep tk <= tq
                    for h in range(2):
                        lo = h * 512 + k * 128
                        nc.vector.tensor_mul(
                            att[:, lo:lo + 128], att[:, lo:lo + 128], mask_sb[:]
                        )
                atts.append(att)

            def emit_av_mms(hp, J, jj, h, av, atts):
                jq = 4 * J + jj
                for i in range(jq + 1):
                    lhsT = atts[i][:, h * 512 + jj * 128:h * 512 + (jj + 1) * 128]
                    hl = hp * 2 + h
                    nc.tensor.matmul(
                        av[:, h * 65:(h + 1) * 65],
                        lhsT,
                        vaug_sb[i][:, hl * 65:(hl + 1) * 65],
                        start=(i == 0), stop=(i == jq),
                    )

            def emit_av_finish(hp, J, jj, av):
                onorm = onp.tile([128, 128], BF16, tag="on", name="on")
                for h in range(2):
                    rc = rcp.tile([128, 1], F32, tag="rc", name="rc")
                    nc.vector.reciprocal(rc[:], av[:, h * 65 + 64:h * 65 + 65])
                    nc.vector.tensor_scalar_mul(
                        onorm[:, h * 64:(h + 1) * 64],
                        av[:, h * 65:h * 65 + 64],
                        rc[:],
                    )
                # transpose scratch folded into the av bank (bf16 view of
                # the same PSUM tile; av is fully consumed by the reads
                # above)
                tp = av.bitcast(BF16)[:, 0:128]
                nc.tensor.transpose(tp, onorm[:], ident[:])
                nc.vector.tensor_copy(
                    outT_sb[hp][:, J * 512 + jj * 128:J * 512 + (jj + 1) * 128],
                    tp,
                )

            def emit_ag(seg):
                hps, lo, hi = ag_bundles[seg]
                w = hi - lo
                for n, hp in enumerate(hps):
                    nc.gpsimd.dma_start(
                        ag_in[seg][:, n * w:(n + 1) * w], outT_sb[hp][:, lo:hi])
                nc.gpsimd.collective_compute(
                    "AllGather",
                    mybir.AluOpType.bypass,
                    replica_groups=[[0, 1], [2, 3], [4, 5], [6, 7]],
                    ins=[ag_in[seg].opt()],
                    outs=[ag_out[seg].opt()],
                )

            def make_av_items(hp, J, atts):
                items = []
                for jj in range(4):
                    av = avpsp.tile([128, 130], F32, tag="av", name="av")
                    for h in range(2):
                        items.append(
                            lambda hp=hp, J=J, jj=jj, h=h, av=av, atts=atts:
                            emit_av_mms(hp, J, jj, h, av, atts)
                        )
                    items.append(
                        lambda hp=hp, J=J, jj=jj, av=av:
                        emit_av_finish(hp, J, jj, av)
                    )
                return items

            def emit_av_self(hp, J, jj, atts):
                av = avpsp.tile([128, 130], F32, tag="av", name="av")
                for h in range(2):
                    emit_av_mms(hp, J, jj, h, av, atts)
                emit_av_finish(hp, J, jj, av)

            # ---- O-projection as per-t-tile units ----
            ag_cur = [None] * ND

            def load_ag_grp(grp, ct):
                """Load ag columns [grp*512, (grp+1)*512) for channel tile
                ct into the rotating slot."""
                t = agsbp.tile([128, TQ], BF16, tag=f"ag{ct}", name=f"ag{ct}")
                hp = ct % NCT
                rows = slice(0, 128) if ct < NCT else slice(128, 256)
                lo = grp * TQ
                # find the bundle holding [lo, lo+TQ) for this hp
                for s, (hps, slo, shi) in ag_bundles.items():
                    if hp in hps and slo <= lo and lo + TQ <= shi:
                        n = hps.index(hp)
                        w = shi - slo
                        eng = nc.scalar if grp == 3 else nc.sync
                        eng.dma_start(
                            t[:],
                            ag_out[s][rows, n * w + (lo - slo):
                                      n * w + (lo - slo) + TQ])
                        break
                else:
                    raise AssertionError(f"no bundle for grp={grp} ct={ct}")
                return t

            PART_CTS = [0, 4, 1, 5, 2, 6]  # hp 0-2: gathered before the tail
            TAIL_CTS = [3, 7]              # hp 3: gated by the last AG

            def oproj_unit(tt):
                yps = accp.tile([128, 512], F32, tag="acc", name="accy")
                th = tt % 4
                for cti in range(ND):
                    nc.tensor.matmul(
                        yps[:],
                        ag_cur[cti][:, th * 128:(th + 1) * 128],
                        wo_sb[:, cti * 512:(cti + 1) * 512],
                        start=(cti == 0), stop=(cti == ND - 1),
                    )
                ysb = ysbp.tile([128, 512], BF16, tag="ysb", name="ysb")
                nc.vector.tensor_copy(ysb[:], yps[:])
                nc.sync.dma_start(y[tt * 128:(tt + 1) * 128, :], ysb[:])

            ypart_sb = [None] * 4

            def oproj_partial(tt):
                """Accumulate the 6 non-hp3 channel tiles of a grp3 unit
                into SBUF while the last AllGather is in flight. Uses the
                st pool (idle after the last scores tile) so the tail
                pipeline has 4 accumulation slots instead of 2."""
                yps = stpsp.tile([128, 1024], F32, tag="st", name="stp")[:, 0:512]
                th = tt % 4
                for n, cti in enumerate(PART_CTS):
                    nc.tensor.matmul(
                        yps[:],
                        ag3[cti][:, th * 128:(th + 1) * 128],
                        wo_sb[:, cti * 512:(cti + 1) * 512],
                        start=(n == 0), stop=(n == len(PART_CTS) - 1),
                    )
                t = ypartp.tile([128, 512], BF16, tag=f"yp{th}", name=f"yp{th}")
                nc.vector.tensor_copy(t[:], yps[:])
                ypart_sb[th] = t

            def oproj_final(tt):
                if tt % 2 == 0:
                    yps = stpsp.tile([128, 1024], F32, tag="st", name="stf")[:, 0:512]
                else:
                    yps = accp.tile([128, 512], F32, tag="acc", name="accf")
                th = tt % 4
                for n, cti in enumerate(TAIL_CTS):
                    nc.tensor.matmul(
                        yps[:],
                        ag3[cti][:, th * 128:(th + 1) * 128],
                        wo_sb[:, cti * 512:(cti + 1) * 512],
                        start=(n == 0), stop=(n == len(TAIL_CTS) - 1),
                    )
                ysb = ysbp.tile([128, 512], BF16, tag="ysb", name="ysb")
                nc.vector.tensor_add(ysb[:], yps[:], ypart_sb[th][:])
                nc.sync.dma_start(y[tt * 128:(tt + 1) * 128, :], ysb[:])

            # O-proj fillers: (deadline, closure). Group g covers t-tiles
            # 4g..4g+3; loads precede units at equal deadlines (stable
            # sort). grp3: non-hp3 loads land in idx 15 after grp2 units;
            # partial/tail work runs post-loop.
            def load_ag_cur(grp, ct):
                ag_cur[ct] = load_ag_grp(grp, ct)

            ag3 = [None] * ND

            def load_ag3(ct):
                ag3[ct] = load_ag_grp(3, ct)

            oitems = []
            for ct in range(ND):
                oitems.append((10, lambda ct=ct: load_ag_cur(0, ct)))
            for tt in range(0, 4):
                oitems.append((12 + tt // 2, lambda tt=tt: oproj_unit(tt)))
            for ct in range(ND):
                oitems.append((13, lambda ct=ct: load_ag_cur(1, ct)))
            for tt in range(4, 8):
                oitems.append((14 + (tt - 4) // 2, lambda tt=tt: oproj_unit(tt)))
            for ct in range(ND):
                oitems.append((15, lambda ct=ct: load_ag_cur(2, ct)))
            oitems.sort(key=lambda x: x[0])

            work = [(J, hp) for J in range(NJ) for hp in range(NCT)]
            n_work = len(work)
            av_queue = []
            fillers.sort(key=lambda x: x[0])
            fpos = 0
            opos = 0
            for idx, (J, hp) in enumerate(work):
                selfdrain = idx >= n_work - 2
                n_tk = 4 * J + 4
                atts = []
                due = []
                while fpos < len(fillers) and fillers[fpos][0] <= idx + 1:
                    due.append(fillers[fpos][1])
                    fpos += 1
                while opos < len(oitems) and oitems[opos][0] <= idx:
                    due.append(oitems[opos][1])
                    opos += 1
                mixed = []
                na, nd_ = len(av_queue), len(due)
                ai = di = 0
                for s in range(na + nd_):
                    if ai * nd_ <= di * na and ai < na:
                        mixed.append(av_queue[ai]); ai += 1
                    elif di < nd_:
                        mixed.append(due[di]); di += 1
                    else:
                        mixed.append(av_queue[ai]); ai += 1
                total = len(mixed)
                done = 0
                for i in range(n_tk):
                    emit_qk_tile(hp, J, i, atts)
                    want = ((i + 1) * total) // n_tk
                    while done < want:
                        mixed[done]()
                        done += 1
                    # self-drain: AV(jj) emitted one tile after its last
                    # att (one-tile slack for the exp pipeline)
                    if selfdrain and i >= n_tk - 3:
                        emit_av_self(hp, J, i - (n_tk - 3), atts)
                while done < total:
                    mixed[done]()
                    done += 1
                if selfdrain:
                    emit_av_self(hp, J, 3, atts)
                    emit_ag("C012" if idx == n_work - 2 else "C3")
                    av_queue = []
                else:
                    av_queue = make_av_items(hp, J, atts)
                # bundled AG issue points (staging DMAs wait on the outT
                # writes via semaphores, so issuing at iteration end of the
                # last contributing drain is safe)
                if idx == 6:
                    emit_ag("A01")
                elif idx == 8:
                    emit_ag("A23")
                elif idx == 12:
                    emit_ag("B")
            for c in av_queue:
                c()
            # tail: all grp3 loads issue up front on the sync queue (PART
            # ones fire immediately off the landed C012; TAIL ones wait on
            # C3), then grp2 units + grp3 partials interleave while C3 is
            # in flight; the 2-matmul finishes run last.
            for ct in PART_CTS:
                load_ag3(ct)
            for ct in TAIL_CTS:
                load_ag3(ct)
            for tt in range(12, 16):
                oproj_partial(tt)
                oproj_unit(tt - 4)
            for tt in range(12, 16):
                oproj_final(tt)

    nc.compile()
    return nc


_NC_CACHE = {}


def _get_nc(T):
    if T not in _NC_CACHE:
        _NC_CACHE[T] = build_nc(T)
    return _NC_CACHE[T]


def _pack_w(w):
    """[1024, 512] (in-dim major) -> [128, 8*512] d-tile-major columns."""
    return np.ascontiguousarray(
        w.reshape(8, 128, 512).transpose(1, 0, 2).reshape(128, 8 * 512))


def _pack_x(xT):
    """[1024, T] -> [128, NJ*8*512] chunk-major, d-tile-minor."""
    T = xT.shape[1]
    nj = T // 512
    return np.ascontiguousarray(
        xT.reshape(8, 128, nj, 512).transpose(1, 2, 0, 3).reshape(128, nj * 8 * 512))


def shard_inputs(x, W_q, W_k, W_v, W_o):
    """Host-side sharding: per-core input dicts (bf16, packed)."""
    tri = np.triu(np.ones((128, 128), np.float32)).astype(NPBF16)
    xp_cache = {}
    in_maps = []
    for c in range(N_CORES):
        b, hg = c // 2, c % 2
        cs = slice(hg * CL, (hg + 1) * CL)
        if b not in xp_cache:
            xp_cache[b] = _pack_x(np.ascontiguousarray(x[b].T).astype(NPBF16))
        in_maps.append({
            "xp": xp_cache[b],
            "wqp": _pack_w(np.ascontiguousarray(W_q[cs, :].T).astype(NPBF16)),
            "wkp": _pack_w(np.ascontiguousarray(W_k[cs, :].T).astype(NPBF16)),
            "wvp": _pack_w(np.ascontiguousarray(W_v[cs, :].T).astype(NPBF16)),
            "wop": _pack_w(np.ascontiguousarray(W_o[cs, :].T).astype(NPBF16)),
            "mask": tri,
        })
    return in_maps


def assemble_output(results, T):
    y = np.zeros((B, T, D), np.float32)
    for c in range(N_CORES):
        b, hg = c // 2, c % 2
        y[b][:, hg * CL:(hg + 1) * CL] = results[c]["y"].astype(np.float32)
    return y


def kernel(x, W_q, W_k, W_v, W_o, _trace=False):
    x = np.asarray(x, dtype=np.float32)
    W_q = np.asarray(W_q, dtype=np.float32)
    W_k = np.asarray(W_k, dtype=np.float32)
    W_v = np.asarray(W_v, dtype=np.float32)
    W_o = np.asarray(W_o, dtype=np.float32)
    T = x.shape[1]
    nc = _get_nc(T)
    in_maps = shard_inputs(x, W_q, W_k, W_v, W_o)
    res = run_bass_kernel_spmd(
        nc, in_maps, core_ids=list(range(N_CORES)), trace=_trace
    )
    out = assemble_output(res.results, T)
    if _trace:
        return out, res
    return out
